# revision 1
# baseline (speedup 1.0000x reference)
"""Trainium2 Bass kernel for nn_BodyAvgDiseaseFeatureAttn2.

Computation (reference):
    attn  = softmax over channels of [heart(27); lung(28); lung(28)] -> [83, 16]
    Weff[o,c,h,w] = attn[o,c] * Wfc[o,c,h,w]
    out[b,o] = mean_s( sum_{c,h,w} x[b,s,c,h,w] * Weff[o,c,h,w] ) + bias[o]

Kernel strategy (pure data parallel, 8 cores, batch-sharded):
  per core (512 volumes):
    - replicate tiny weights; compute softmax/15 and Weff on-device
    - per 128-volume tile: DMA x [128, 8640]; DVE slice-sum -> [128, 576];
      PE-transpose -> [576, 128]; PE matmul with Weff^T -> psum [83, 128];
      bias add -> out sbuf [83, 512]
    - one DMA of [83, 512] out; host assembles [4096, 83]
"""

import numpy as np
from contextlib import ExitStack

import concourse.bass as bass
import concourse.bacc as bacc
import concourse.tile as tile
import concourse.mybir as mybir
from concourse import masks
from concourse.bass_utils import run_bass_kernel_spmd

F32 = mybir.dt.float32
AX = mybir.AxisListType
OP = mybir.AluOpType
ACT = mybir.ActivationFunctionType

N_CORES = 8
B, S, C, H, W = 4096, 15, 16, 6, 6
CK = C * H * W            # 576
SCK = S * CK              # 8640
NH, NL = 27, 28
O = 2 * NL + NH           # 83
BS = B // N_CORES         # 512 volumes per core
P = 128                   # partition tile
NT = BS // P              # 4 batch tiles per core
SH = 8                    # slices in first DMA chunk (second gets 7)
KC = [128, 128, 128, 128, 64]  # contraction chunking of 576


def _build_body(ctx, tc, o_d, x_d, h_d, l_d, w_d, b_d):
    nc = tc.nc

    const = ctx.enter_context(tc.tile_pool(name="const", bufs=1))
    ptr = ctx.enter_context(tc.tile_pool(name="ptr", bufs=4, space="PSUM"))
    pout = ctx.enter_context(tc.tile_pool(name="pout", bufs=2, space="PSUM"))
    xpool = ctx.enter_context(tc.tile_pool(name="xp", bufs=4))
    spool = ctx.enter_context(tc.tile_pool(name="sp", bufs=4))
    xtp = ctx.enter_context(tc.tile_pool(name="xtp", bufs=2))

    # ---- constants / setup --------------------------------------------
    ident = const.tile([128, 128], F32)
    masks.make_identity(nc, ident[:])

    attn = const.tile([O, 16], F32)
    nc.sync.dma_start(attn[0:NH, :], h_d[:, :])
    nc.sync.dma_start(attn[NH:NH + NL, :], l_d[:, :])
    nc.sync.dma_start(attn[NH + NL:O, :], l_d[:, :])

    wsb = const.tile([O, CK], F32)
    nc.sync.dma_start(wsb[:], w_d[:, :])
    bias = const.tile([O, 1], F32)
    nc.sync.dma_start(bias[:], b_d[:, :])

    # softmax over the 16 channels, folded with the 1/S slice-average
    negmax = const.tile([O, 1], F32)
    nc.vector.tensor_reduce(negmax[:], attn[:], axis=AX.X, op=OP.max, negate=True)
    att_e = const.tile([O, 16], F32)
    den = const.tile([O, 1], F32)
    nc.scalar.activation(att_e[:], attn[:], ACT.Exp, bias=negmax[:, :], scale=1.0,
                         accum_out=den[:])
    den_s = const.tile([O, 1], F32)
    nc.scalar.mul(den_s[:], den[:], float(S))
    rden = const.tile([O, 1], F32)
    nc.vector.reciprocal(rden[:], den_s[:])
    attn_n = const.tile([O, 16], F32)
    nc.vector.tensor_scalar_mul(attn_n[:], att_e[:], rden[:, :])

    # Weff[o, c, k] = attn_n[o, c] * Wfc[o, c, k]   (k = 36 spatial)
    weff = const.tile([O, CK], F32)
    w_v = wsb[:].rearrange("p (c k) -> p c k", c=C)
    a_v = attn_n[:].rearrange("p (c k) -> p c k", k=1)
    o_v = weff[:].rearrange("p (c k) -> p c k", c=C)
    w_bc, a_bc = bass.broadcast_tensor_aps(w_v, a_v)
    nc.vector.tensor_tensor(o_v, w_bc, a_bc, op=OP.mult)

    # Weff^T chunks: wT[:, k*O:(k+1)*O] holds Weff[:, k-chunk].T  ([kw, 83])
    wT = const.tile([128, 5 * O], F32)
    for k, kw in enumerate(KC):
        c0 = 128 * k
        pt = ptr.tile([128, 128], F32, tag="pt")
        nc.tensor.transpose(pt[0:kw, 0:O], weff[:, c0:c0 + kw], ident[0:O, 0:O])
        nc.scalar.copy(wT[0:kw, k * O:(k + 1) * O], pt[0:kw, 0:O])

    outsb = const.tile([O, BS], F32)

    # ---- main loop over batch tiles -----------------------------------
    for t in range(NT):
        b0 = t * P
        xa = xpool.tile([P, SH * CK], F32, tag="xh")
        nc.sync.dma_start(xa[:], x_d[b0:b0 + P, 0:SH * CK])
        xb = xpool.tile([P, (S - SH) * CK], F32, tag="xh")
        nc.sync.dma_start(xb[:], x_d[b0:b0 + P, SH * CK:SCK])

        sa = spool.tile([P, CK], F32, tag="s")
        nc.vector.tensor_reduce(sa[:], xa[:].rearrange("p (s c) -> p c s", s=SH),
                                axis=AX.X, op=OP.add)
        sb = spool.tile([P, CK], F32, tag="s")
        nc.vector.tensor_reduce(sb[:], xb[:].rearrange("p (s c) -> p c s", s=S - SH),
                                axis=AX.X, op=OP.add)
        ss = spool.tile([P, CK], F32, tag="ss")
        nc.vector.tensor_add(ss[:], sa[:], sb[:])

        # transpose slice-sums: xT[:, k*P:(k+1)*P] = ss[:, k-chunk].T  ([kw, 128])
        xT = xtp.tile([128, 5 * P], F32)
        for k, kw in enumerate(KC):
            c0 = 128 * k
            pt = ptr.tile([128, 128], F32, tag="pt")
            nc.tensor.transpose(pt[0:kw, :], ss[:, c0:c0 + kw], ident[:, :])
            nc.scalar.copy(xT[0:kw, k * P:(k + 1) * P], pt[0:kw, :])

        po = pout.tile([O, P], F32)
        for k, kw in enumerate(KC):
            nc.tensor.matmul(po[:], wT[0:kw, k * O:(k + 1) * O],
                             xT[0:kw, k * P:(k + 1) * P],
                             start=(k == 0), stop=(k == len(KC) - 1))

        nc.vector.tensor_scalar_add(outsb[:, b0:b0 + P], po[:], bias[:, :])

    nc.sync.dma_start(o_d[:, :], outsb[:])


def build_program(repeat: int = 1):
    nc = bacc.Bacc("TRN2", target_bir_lowering=False, debug=False,
                   num_devices=N_CORES)
    x_d = nc.dram_tensor("x", [BS, SCK], F32, kind="ExternalInput").ap()
    h_d = nc.dram_tensor("heart", [NH, 16], F32, kind="ExternalInput").ap()
    l_d = nc.dram_tensor("lung", [NL, 16], F32, kind="ExternalInput").ap()
    w_d = nc.dram_tensor("fcw", [O, CK], F32, kind="ExternalInput").ap()
    b_d = nc.dram_tensor("fcb", [O, 1], F32, kind="ExternalInput").ap()
    o_d = nc.dram_tensor("out", [O, BS], F32, kind="ExternalOutput").ap()

    with tile.TileContext(nc) as tc:
        if repeat == 1:
            with ExitStack() as ctx:
                _build_body(ctx, tc, o_d, x_d, h_d, l_d, w_d, b_d)
        else:
            def body(_iv):
                with ExitStack() as ctx:
                    _build_body(ctx, tc, o_d, x_d, h_d, l_d, w_d, b_d)
            tc.For_i_unrolled(0, repeat, 1, body, max_unroll=1)
    nc.compile()
    return nc


_NC_CACHE = {}


def _get_program(repeat: int = 1):
    if repeat not in _NC_CACHE:
        _NC_CACHE[repeat] = build_program(repeat)
    return _NC_CACHE[repeat]


def make_in_maps(inputs):
    x = np.asarray(inputs["x"], dtype=np.float32).reshape(B, SCK)
    h = np.asarray(inputs["dzfeatweights_heart"], dtype=np.float32).reshape(NH, 16)
    l = np.asarray(inputs["dzfeatweights_lung"], dtype=np.float32).reshape(NL, 16)
    w = np.asarray(inputs["fclayers_weights"], dtype=np.float32).reshape(O, CK)
    b = np.asarray(inputs["fclayers_biases"], dtype=np.float32).reshape(O, 1)
    return [{"x": x[c * BS:(c + 1) * BS], "heart": h, "lung": l, "fcw": w, "fcb": b}
            for c in range(N_CORES)]


def assemble_output(results):
    outs = [results[c]["out"] for c in range(N_CORES)]    # each [83, 512]
    return np.ascontiguousarray(np.concatenate(outs, axis=1).T)  # [4096, 83]


def kernel(**inputs) -> np.ndarray:
    nc = _get_program(1)
    res = run_bass_kernel_spmd(nc, make_in_maps(inputs), list(range(N_CORES)))
    return assemble_output(res.results)


# revision 8
# speedup vs baseline: 2.4976x; 2.4976x over previous
"""Trainium2 Bass kernel for nn_BodyAvgDiseaseFeatureAttn2.

Computation (reference):
    attn  = softmax over channels of [heart(27); lung(28); lung(28)] -> [83, 16]
    Weff[o,c,h,w] = attn[o,c] * Wfc[o,c,h,w]
    out[b,o] = mean_s( sum_{c,h,w} x[b,s,c,h,w] * Weff[o,c,h,w] ) + bias[o]

Kernel strategy (pure data parallel, 8 cores, batch-sharded):
  per core (512 volumes):
    - replicate tiny weights; compute softmax/15 and Weff on-device
    - per 128-volume tile: DMA x [128, 8640]; DVE slice-sum -> [128, 576];
      PE-transpose -> [576, 128]; PE matmul with Weff^T -> psum [83, 128];
      bias add -> out sbuf [83, 512]
    - one DMA of [83, 512] out; host assembles [4096, 83]
"""

import numpy as np
from contextlib import ExitStack

import concourse.bass as bass
import concourse.bacc as bacc
import concourse.tile as tile
import concourse.mybir as mybir
from concourse import masks
from concourse.bass_utils import run_bass_kernel_spmd

F32 = mybir.dt.float32
AX = mybir.AxisListType
OP = mybir.AluOpType
ACT = mybir.ActivationFunctionType

N_CORES = 8
B, S, C, H, W = 4096, 15, 16, 6, 6
CK = C * H * W            # 576
SCK = S * CK              # 8640
NH, NL = 27, 28
O = 2 * NL + NH           # 83
BS = B // N_CORES         # 512 volumes per core
P = 128                   # partition tile
NT = BS // P              # 4 batch tiles per core
SH = 8                    # slices in first DMA chunk (second gets 7)
KC = [128, 128, 128, 128, 64]  # contraction chunking of 576


def _build_body(ctx, tc, o_d, x_d, h_d, l_d, w_d, b_d):
    nc = tc.nc

    const = ctx.enter_context(tc.tile_pool(name="const", bufs=1))
    ptr = ctx.enter_context(tc.tile_pool(name="ptr", bufs=4, space="PSUM"))
    pout = ctx.enter_context(tc.tile_pool(name="pout", bufs=2, space="PSUM"))
    xpool = ctx.enter_context(tc.tile_pool(name="xp", bufs=10))
    spool = ctx.enter_context(tc.tile_pool(name="sp", bufs=8))
    hpool = ctx.enter_context(tc.tile_pool(name="hp", bufs=3))
    xtp = ctx.enter_context(tc.tile_pool(name="xtp", bufs=2))

    # ---- constants / setup --------------------------------------------
    ident = const.tile([128, 128], F32)
    masks.make_identity(nc, ident[:])

    attn = const.tile([O, 16], F32)
    nc.sync.dma_start(attn[0:NH, :], h_d[:, :])
    nc.sync.dma_start(attn[NH:NH + NL, :], l_d[:, :])
    nc.sync.dma_start(attn[NH + NL:O, :], l_d[:, :])

    wsb = const.tile([O, CK], F32)
    nc.sync.dma_start(wsb[:], w_d[:, :])
    bias = const.tile([O, 1], F32)
    nc.sync.dma_start(bias[:], b_d[:, :])

    # softmax over the 16 channels, folded with the 1/S slice-average
    negmax = const.tile([O, 1], F32)
    nc.vector.tensor_reduce(negmax[:], attn[:], axis=AX.X, op=OP.max, negate=True)
    att_e = const.tile([O, 16], F32)
    den = const.tile([O, 1], F32)
    nc.scalar.activation(att_e[:], attn[:], ACT.Exp, bias=negmax[:, :], scale=1.0,
                         accum_out=den[:])
    den_s = const.tile([O, 1], F32)
    nc.scalar.mul(den_s[:], den[:], float(S))
    rden = const.tile([O, 1], F32)
    nc.vector.reciprocal(rden[:], den_s[:])
    attn_n = const.tile([O, 16], F32)
    nc.vector.tensor_scalar_mul(attn_n[:], att_e[:], rden[:, :])

    # Weff[o, c, k] = attn_n[o, c] * Wfc[o, c, k]   (k = 36 spatial)
    weff = const.tile([O, CK], F32)
    w_v = wsb[:].rearrange("p (c k) -> p c k", c=C)
    a_v = attn_n[:].rearrange("p (c k) -> p c k", k=1)
    o_v = weff[:].rearrange("p (c k) -> p c k", c=C)
    w_bc, a_bc = bass.broadcast_tensor_aps(w_v, a_v)
    nc.vector.tensor_tensor(o_v, w_bc, a_bc, op=OP.mult)

    # Weff^T chunks: wT[:, k*O:(k+1)*O] holds Weff[:, k-chunk].T  ([kw, 83])
    wT = const.tile([128, 5 * O], F32)
    for k, kw in enumerate(KC):
        c0 = 128 * k
        pt = ptr.tile([128, 128], F32, tag="pt")
        nc.tensor.transpose(pt[0:kw, 0:O], weff[:, c0:c0 + kw], ident[0:O, 0:O])
        nc.scalar.copy(wT[0:kw, k * O:(k + 1) * O], pt[0:kw, 0:O])

    outsb = const.tile([O, BS], F32)

    # ---- main loop over batch tiles -----------------------------------
    # x tile is loaded in 4 slice-aligned chunks: 4+4+4+3 slices.
    QS = [(0, 4), (4, 4), (8, 4), (12, 3)]
    for t in range(NT):
        b0 = t * P
        qs = []
        for (s0, ns) in QS:
            xq = xpool.tile([P, 4 * CK], F32, tag="xq")
            nc.sync.dma_start(xq[:, 0:ns * CK],
                             x_d[b0:b0 + P, s0 * CK:(s0 + ns) * CK])
            qs.append(xq)

        # contiguous pairwise tree slice-sum, split DVE / GPSIMD:
        #   DVE:  h0 h1 r0 r1 u      GPSIMD: h2 h3 r2 r3 v      DVE: ss
        h0 = hpool.tile([P, 2 * CK], F32, tag="h")
        nc.vector.tensor_add(h0[:], qs[0][:, 0:2 * CK], qs[0][:, 2 * CK:4 * CK])
        h1 = hpool.tile([P, 2 * CK], F32, tag="h")
        nc.vector.tensor_add(h1[:], qs[1][:, 0:2 * CK], qs[1][:, 2 * CK:4 * CK])
        h2 = hpool.tile([P, 2 * CK], F32, tag="h")
        nc.gpsimd.tensor_add(h2[:], qs[2][:, 0:2 * CK], qs[2][:, 2 * CK:4 * CK])
        h3 = spool.tile([P, CK], F32, tag="s")
        nc.gpsimd.tensor_add(h3[:], qs[3][:, 0:CK], qs[3][:, CK:2 * CK])
        r0 = spool.tile([P, CK], F32, tag="s")
        nc.vector.tensor_add(r0[:], h0[:, 0:CK], h0[:, CK:2 * CK])
        r1 = spool.tile([P, CK], F32, tag="s")
        nc.vector.tensor_add(r1[:], h1[:, 0:CK], h1[:, CK:2 * CK])
        r2 = spool.tile([P, CK], F32, tag="s")
        nc.gpsimd.tensor_add(r2[:], h2[:, 0:CK], h2[:, CK:2 * CK])
        r3 = spool.tile([P, CK], F32, tag="s")
        nc.gpsimd.tensor_add(r3[:], h3[:], qs[3][:, 2 * CK:3 * CK])
        u = spool.tile([P, CK], F32, tag="s")
        nc.vector.tensor_add(u[:], r0[:], r1[:])
        v = spool.tile([P, CK], F32, tag="s")
        nc.gpsimd.tensor_add(v[:], r2[:], r3[:])
        ss = spool.tile([P, CK], F32, tag="ss")
        nc.vector.tensor_add(ss[:], u[:], v[:])

        # PE transpose the slice-sum: xT[:, k*P:(k+1)*P] = ss[:, chunk].T
        xT = xtp.tile([128, 5 * P], F32)
        for k, kw in enumerate(KC):
            c0 = 128 * k
            pt = ptr.tile([128, 128], F32, tag="pt")
            nc.tensor.transpose(pt[0:kw, :], ss[:, c0:c0 + kw], ident[:, :])
            nc.scalar.copy(xT[0:kw, k * P:(k + 1) * P], pt[0:kw, :])

        po = pout.tile([O, P], F32)
        for k, kw in enumerate(KC):
            nc.tensor.matmul(po[:], wT[0:kw, k * O:(k + 1) * O],
                             xT[0:kw, k * P:(k + 1) * P],
                             start=(k == 0), stop=(k == len(KC) - 1))

        nc.vector.tensor_scalar_add(outsb[:, b0:b0 + P], po[:], bias[:, :])

    nc.sync.dma_start(o_d[:, :], outsb[:])


def build_program(repeat: int = 1):
    nc = bacc.Bacc("TRN2", target_bir_lowering=False, debug=False,
                   num_devices=N_CORES)
    x_d = nc.dram_tensor("x", [BS, SCK], F32, kind="ExternalInput").ap()
    h_d = nc.dram_tensor("heart", [NH, 16], F32, kind="ExternalInput").ap()
    l_d = nc.dram_tensor("lung", [NL, 16], F32, kind="ExternalInput").ap()
    w_d = nc.dram_tensor("fcw", [O, CK], F32, kind="ExternalInput").ap()
    b_d = nc.dram_tensor("fcb", [O, 1], F32, kind="ExternalInput").ap()
    o_d = nc.dram_tensor("out", [O, BS], F32, kind="ExternalOutput").ap()

    with tile.TileContext(nc) as tc:
        if repeat == 1:
            with ExitStack() as ctx:
                _build_body(ctx, tc, o_d, x_d, h_d, l_d, w_d, b_d)
        else:
            def body(_iv):
                with ExitStack() as ctx:
                    _build_body(ctx, tc, o_d, x_d, h_d, l_d, w_d, b_d)
            tc.For_i_unrolled(0, repeat, 1, body, max_unroll=1)
    nc.compile()
    return nc


_NC_CACHE = {}


def _get_program(repeat: int = 1):
    if repeat not in _NC_CACHE:
        _NC_CACHE[repeat] = build_program(repeat)
    return _NC_CACHE[repeat]


def make_in_maps(inputs):
    x = np.asarray(inputs["x"], dtype=np.float32).reshape(B, SCK)
    h = np.asarray(inputs["dzfeatweights_heart"], dtype=np.float32).reshape(NH, 16)
    l = np.asarray(inputs["dzfeatweights_lung"], dtype=np.float32).reshape(NL, 16)
    w = np.asarray(inputs["fclayers_weights"], dtype=np.float32).reshape(O, CK)
    b = np.asarray(inputs["fclayers_biases"], dtype=np.float32).reshape(O, 1)
    return [{"x": x[c * BS:(c + 1) * BS], "heart": h, "lung": l, "fcw": w, "fcb": b}
            for c in range(N_CORES)]


def assemble_output(results):
    outs = [results[c]["out"] for c in range(N_CORES)]    # each [83, 512]
    return np.ascontiguousarray(np.concatenate(outs, axis=1).T)  # [4096, 83]


def kernel(**inputs) -> np.ndarray:
    nc = _get_program(1)
    res = run_bass_kernel_spmd(nc, make_in_maps(inputs), list(range(N_CORES)))
    return assemble_output(res.results)
